# revision 2
# baseline (speedup 1.0000x reference)
"""Trainium2 Bass kernel for nn_ByteGridModel (dense_cnn).

Sharding: pure data-parallel over batch B=8 -> 8 cores, one batch item per
core, no collectives. Weights replicated (streamed per layer, double
buffered).

Per-core layout: channels on partitions, h = [H=512 -> 4x128, S=256] fp32
resident in SBUF.

Per layer:
  - rmsnorm: ACT square -> fp32r ones-matmul partition reduction -> sqrt ->
    DVE reciprocal -> fp32r broadcast matmul -> DVE multiply (bf16 out).
    rms weights / alphas are folded into the mixer/GLU weights on host.
  - per-channel 16x16 mixers: DVE/GPSIMD broadcast-AP products, accumulated
    on PE with identity matmuls into PSUM.
  - GLU MLP: bf16 PE matmuls (Wv/Wg/Wo), Silu on ACT, gate-mul on DVE.
"""

import numpy as np
import ml_dtypes

import concourse.bacc as bacc
import concourse.bass as bass
import concourse.tile as tile
import concourse.mybir as mybir
from concourse.bass_utils import run_bass_kernel_spmd

B, S, H, GLU, VOC, L, CIN, BLK = 8, 256, 512, 1024, 256, 24, 320, 16
EPS = 1e-5
NT = H // 128  # 4 channel tiles
GT = GLU // 128  # 8 glu tiles

F32 = mybir.dt.float32
F32R = mybir.dt.float32r
BF16 = mybir.dt.bfloat16
MULT = mybir.AluOpType.mult
ADD = mybir.AluOpType.add
AF = mybir.ActivationFunctionType

_PROG_CACHE = {}


def _bview(base, doff, free_dims):
    """View of a 2D sbuf AP with custom (possibly broadcast) free dims."""
    return bass.AP(
        tensor=base.tensor,
        offset=base.offset + doff,
        ap=[list(base.ap[0])] + [list(d) for d in free_dims],
    )


def build_program(n_layers=L, sim_compat=False):
    nc = bacc.Bacc("TRN2", enable_partition_id=False)

    x_d = nc.dram_tensor("x", [384, S], F32R, kind="ExternalInput")
    stw_d = nc.dram_tensor("stem_wT", [384, H], F32R, kind="ExternalInput")
    wv_d = nc.dram_tensor("wvT", [n_layers, H, GLU], BF16, kind="ExternalInput")
    wg_d = nc.dram_tensor("wgT", [n_layers, H, GLU], BF16, kind="ExternalInput")
    wo_d = nc.dram_tensor("woT", [n_layers, GLU, H], BF16, kind="ExternalInput")
    wl_d = nc.dram_tensor("wl", [n_layers, H, 256], BF16, kind="ExternalInput")
    wm_d = nc.dram_tensor("wm", [n_layers, H, 256], BF16, kind="ExternalInput")
    hw_d = nc.dram_tensor("headT", [H, VOC], BF16, kind="ExternalInput")
    id_d = nc.dram_tensor("ident", [128, 128], BF16, kind="ExternalInput")
    ones_d = nc.dram_tensor("ones_k", [128, 1], F32R, kind="ExternalInput")
    onesr_d = nc.dram_tensor("ones_m", [1, 128], F32R, kind="ExternalInput")
    out_d = nc.dram_tensor("out", [VOC, S], F32, kind="ExternalOutput")

    from contextlib import ExitStack

    with tile.TileContext(nc) as tc, ExitStack() as ctx:
        singles = ctx.enter_context(tc.tile_pool(name="singles", bufs=1))
        wpool = ctx.enter_context(tc.tile_pool(name="wpool", bufs=2))
        hpool = ctx.enter_context(tc.tile_pool(name="hpool", bufs=1))
        npool = ctx.enter_context(tc.tile_pool(name="npool", bufs=2))
        apool = ctx.enter_context(tc.tile_pool(name="apool", bufs=2))
        ppool = ctx.enter_context(tc.tile_pool(name="ppool", bufs=3))
        gpool = ctx.enter_context(tc.tile_pool(name="gpool", bufs=2))
        ps_n = ctx.enter_context(tc.tile_pool(name="ps_n", bufs=1, space="PSUM"))
        ps_m = ctx.enter_context(tc.tile_pool(name="ps_m", bufs=2, space="PSUM"))
        ps_g = ctx.enter_context(tc.tile_pool(name="ps_g", bufs=2, space="PSUM"))
        ps_o = ctx.enter_context(tc.tile_pool(name="ps_o", bufs=2, space="PSUM"))

        # ---- constants / stem operands ----
        ident = singles.tile([128, 128], BF16, tag="ident")
        nc.sync.dma_start(out=ident, in_=id_d[:])
        ones_k_st = singles.tile([128, 1], F32R, tag="ones_k_st")
        nc.sync.dma_start(out=ones_k_st, in_=ones_d[:])
        ones_k = singles.tile([128, 1], F32R, tag="ones_k")
        ones_m_st = singles.tile([1, 128], F32R, tag="ones_m_st")
        nc.sync.dma_start(out=ones_m_st, in_=onesr_d[:])
        ones_m = singles.tile([1, 128], F32R, tag="ones_m")
        eps_sb = singles.tile([1, 1], F32, tag="eps")
        nc.vector.memset(eps_sb, float(EPS))

        x_st = singles.tile([128, 3, S], F32R, tag="x_st")
        nc.sync.dma_start(out=x_st, in_=x_d[:].rearrange("(t p) s -> p t s", p=128))
        x_sb = singles.tile([128, 3, S], F32R, tag="x")
        stw_st = singles.tile([128, 3, H], F32R, tag="stw_st")
        nc.sync.dma_start(out=stw_st, in_=stw_d[:].rearrange("(t p) s -> p t s", p=128))
        stw_sb = singles.tile([128, 3, H], F32R, tag="stw")

        # Route fp32r matmul operands through a DVE copy so each matmul's
        # operand has an engine writer (a matmul can carry only one
        # cross-engine wait through walrus codegen). Touch bf16 weight DMAs
        # with ldweights for the same reason.
        with nc.allow_low_precision(reason="fp32r staging copies"):
            nc.vector.tensor_copy(out=ones_k, in_=ones_k_st)
            nc.vector.tensor_copy(out=ones_m, in_=ones_m_st)
            nc.vector.tensor_copy(out=x_sb, in_=x_st)
            nc.vector.tensor_copy(out=stw_sb, in_=stw_st)
        nc.tensor.ldweights(ident[:, 0:128])

        # ---- h tiles (resident, fp32) ----
        h = [
            hpool.tile([128, S], F32, tag=f"h{t}", name=f"h{t}") for t in range(NT)
        ]

        # ---- stem: h = stem_w @ x ----
        for t in range(NT):
            pst = ps_o.tile([128, S], F32, tag="po")
            for kt in range(3):
                nc.tensor.matmul(
                    pst,
                    stw_sb[:, kt, t * 128 : (t + 1) * 128],
                    x_sb[:, kt, :],
                    start=(kt == 0),
                    stop=(kt == 2),
                )
            nc.vector.tensor_copy(out=h[t], in_=pst)

        def rms_bcast():
            """Returns PSUM [128, S] fp32 broadcast of 1/sqrt(mean(h^2)+eps)."""
            ms = ps_n.tile([1, S], F32, tag="ms")
            for t in range(NT):
                sq = apool.tile([128, S], F32R, tag="sq")
                nc.scalar.square(sq, h[t])
                nc.tensor.matmul(
                    ms,
                    ones_k[:, 0:1],
                    sq[:],
                    start=(t == 0),
                    stop=(t == NT - 1),
                )
            stdv = npool.tile([1, S], F32, tag="stdv")
            nc.scalar.activation(stdv, ms, AF.Sqrt, bias=eps_sb[0:1, 0:1], scale=1.0 / H)
            rstd = npool.tile([1, S], F32R, tag="rstd")
            with nc.allow_low_precision(reason="fp32r rstd for broadcast matmul"):
                nc.vector.reciprocal(rstd, stdv)
            rb = ps_n.tile([128, S], F32, tag="rb")
            nc.tensor.matmul(
                rb,
                ones_m[0:1, :],
                rstd[:],
                start=True,
                stop=True,
            )
            return rb

        for l in range(n_layers):
            wv_sb = wpool.tile([128, NT, GLU], BF16, tag="wv")
            nc.sync.dma_start(
                out=wv_sb, in_=wv_d[l].rearrange("(t p) o -> p t o", p=128)
            )
            wg_sb = wpool.tile([128, NT, GLU], BF16, tag="wg")
            nc.sync.dma_start(
                out=wg_sb, in_=wg_d[l].rearrange("(t p) o -> p t o", p=128)
            )
            wo_sb = wpool.tile([128, GT, H], BF16, tag="wo")
            nc.sync.dma_start(
                out=wo_sb, in_=wo_d[l].rearrange("(t p) c -> p t c", p=128)
            )
            wl_sb = wpool.tile([128, NT, 256], BF16, tag="wl")
            nc.sync.dma_start(
                out=wl_sb, in_=wl_d[l].rearrange("(t p) q -> p t q", p=128)
            )
            wm_sb = wpool.tile([128, NT, 256], BF16, tag="wm")
            nc.sync.dma_start(
                out=wm_sb, in_=wm_d[l].rearrange("(t p) q -> p t q", p=128)
            )
            nc.tensor.ldweights(wv_sb[:, 0, 0:128])
            nc.tensor.ldweights(wg_sb[:, 0, 0:128])
            nc.tensor.ldweights(wo_sb[:, 0, 0:128])

            # ---------- local mixer: out[c,i,p] = sum_j Wl[c,p,j] u[c,i,j]
            rb = rms_bcast()
            for t in range(NT):
                u = apool.tile([128, S], BF16, tag=f"u{t}")
                nc.vector.tensor_tensor(out=u, in0=h[t], in1=rb, op=MULT)
                acc = ps_m.tile([128, S], F32, tag="macc")
                wbase = wl_sb[:, t, :]
                for g in range(4):  # groups of 4 j values
                    j0 = 4 * g
                    prod = ppool.tile([128, 4, 16, 16], BF16, tag="prod")
                    # u view: (j, i, p) ; broadcast over p
                    uv = _bview(u[:], j0, [[1, 4], [16, 16], [0, 16]])
                    # wl view: free = 16p + j -> (j, i, p); broadcast over i
                    wvv = _bview(wbase, j0, [[1, 4], [0, 16], [16, 16]])
                    eng = nc.gpsimd if g == 3 else nc.vector
                    eng.tensor_tensor(out=prod, in0=uv, in1=wvv, op=MULT)
                    for jj in range(4):
                        nc.tensor.matmul(
                            acc,
                            ident[:],
                            prod[:, jj],
                            start=(g == 0 and jj == 0),
                            stop=(g == 3 and jj == 3),
                        )
                nc.vector.tensor_tensor(out=h[t], in0=h[t], in1=acc, op=ADD)

            # ---------- global mixer: out[c,p,j] = sum_i Wg[c,p,i] v[c,i,j]
            rb = rms_bcast()
            for t in range(NT):
                v = apool.tile([128, S], BF16, tag=f"u{t}")
                nc.vector.tensor_tensor(out=v, in0=h[t], in1=rb, op=MULT)
                acc = ps_m.tile([128, S], F32, tag="macc")
                wbase = wm_sb[:, t, :]
                for g in range(4):  # groups of 4 i values
                    i0 = 4 * g
                    prod = ppool.tile([128, 4, 16, 16], BF16, tag="prod")
                    # v view: (i, p, j); broadcast over p
                    vv = _bview(v[:], 16 * i0, [[16, 4], [0, 16], [1, 16]])
                    # wm view: free = 16p + i -> (i, p, j); broadcast over j
                    wvv = _bview(wbase, i0, [[1, 4], [16, 16], [0, 16]])
                    eng = nc.gpsimd if g == 3 else nc.vector
                    eng.tensor_tensor(out=prod, in0=vv, in1=wvv, op=MULT)
                    for ii in range(4):
                        nc.tensor.matmul(
                            acc,
                            ident[:],
                            prod[:, ii],
                            start=(g == 0 and ii == 0),
                            stop=(g == 3 and ii == 3),
                        )
                nc.vector.tensor_tensor(out=h[t], in0=h[t], in1=acc, op=ADD)

            # ---------- GLU MLP
            rb = rms_bcast()
            wn = []
            for t in range(NT):
                w = apool.tile([128, S], BF16, tag=f"wn{t}")
                nc.vector.tensor_tensor(out=w, in0=h[t], in1=rb, op=MULT)
                wn.append(w)
            gts = []
            for ot in range(GT):
                p1 = ps_g.tile([128, S], F32, tag="pg")
                for kt in range(NT):
                    nc.tensor.matmul(
                        p1,
                        wv_sb[:, kt, ot * 128 : (ot + 1) * 128],
                        wn[kt][:],
                        start=(kt == 0),
                        stop=(kt == NT - 1),
                    )
                s1 = apool.tile([128, S], BF16, tag="s1")
                if sim_compat:
                    # CoreSim has no Silu: emulate with Sigmoid + extra mul
                    sg = apool.tile([128, S], BF16, tag="sg")
                    nc.scalar.activation(sg, p1, AF.Sigmoid)
                    nc.vector.tensor_tensor(out=s1, in0=sg, in1=p1, op=MULT)
                else:
                    nc.scalar.activation(s1, p1, AF.Silu)
                p3 = ps_g.tile([128, S], F32, tag="pg")
                for kt in range(NT):
                    nc.tensor.matmul(
                        p3,
                        wg_sb[:, kt, ot * 128 : (ot + 1) * 128],
                        wn[kt][:],
                        start=(kt == 0),
                        stop=(kt == NT - 1),
                    )
                gt_ = gpool.tile([128, S], BF16, tag=f"g{ot}")
                nc.vector.tensor_tensor(out=gt_, in0=s1, in1=p3, op=MULT)
                gts.append(gt_)
            for t in range(NT):
                po = ps_o.tile([128, S], F32, tag="po")
                for ot in range(GT):
                    nc.tensor.matmul(
                        po,
                        wo_sb[:, ot, t * 128 : (t + 1) * 128],
                        gts[ot][:],
                        start=(ot == 0),
                        stop=(ot == GT - 1),
                    )
                nc.vector.tensor_tensor(out=h[t], in0=h[t], in1=po, op=ADD)

        # ---------- head ----------
        hw_sb = singles.tile([128, NT, VOC], BF16, tag="hw")
        nc.sync.dma_start(out=hw_sb, in_=hw_d.rearrange("(t p) v -> p t v", p=128))
        nc.tensor.ldweights(hw_sb[:, 0, 0:128])
        rb = rms_bcast()
        nrm = []
        for t in range(NT):
            n_ = apool.tile([128, S], BF16, tag=f"wn{t}")
            nc.vector.tensor_tensor(out=n_, in0=h[t], in1=rb, op=MULT)
            nrm.append(n_)
        for mc in range(VOC // 128):
            po = ps_o.tile([128, S], F32, tag="po")
            for kt in range(NT):
                nc.tensor.matmul(
                    po,
                    hw_sb[:, kt, mc * 128 : (mc + 1) * 128],
                    nrm[kt][:],
                    start=(kt == 0),
                    stop=(kt == NT - 1),
                )
            ot_sb = apool.tile([128, S], F32, tag="osb")
            nc.vector.tensor_copy(out=ot_sb, in_=po)
            nc.sync.dma_start(out=out_d[mc * 128 : (mc + 1) * 128, :], in_=ot_sb)

    nc.compile()
    return nc


def _prep_inputs(inputs, n_layers=L):
    """Host-side weight folding + layout prep. Returns dict of np arrays."""
    f = lambda k: np.asarray(inputs[k], dtype=np.float32)
    x = f("x")
    stem_w = f("stem_w")  # [H, CIN]
    rl, rg, rf = f("rms_local"), f("rms_global"), f("rms_ffn")
    al, ag, am = f("alpha_local"), f("alpha_global"), f("alpha_mlp")
    w_local, w_global = f("w_local"), f("w_global")  # [L, H, BLK, BLK]
    wv, wg, wo = f("wv"), f("wg"), f("wo")
    head_rms, head_w = f("head_rms"), f("head_w")
    hls = np.float32(np.asarray(inputs["head_logit_scale"]))

    bf = ml_dtypes.bfloat16
    nl = n_layers

    # local: fold alpha_local * rms_local[c] into Wl[c,p,j]; layout [c, 16p+j]
    wl_h = (w_local[:nl] * al[:nl, None, None, None] * rl[:nl, :, None, None]).reshape(
        nl, H, 256
    )
    # global: Wg[c,p,i]; layout [c, 16p+i]
    wm_h = (w_global[:nl] * ag[:nl, None, None, None] * rg[:nl, :, None, None]).reshape(
        nl, H, 256
    )
    # GLU: fold rms_ffn into wv/wg columns; alpha_mlp into wo
    wvT = np.ascontiguousarray(
        np.transpose(wv[:nl] * rf[:nl, None, :], (0, 2, 1))
    )  # [L, H, GLU]
    wgT = np.ascontiguousarray(np.transpose(wg[:nl] * rf[:nl, None, :], (0, 2, 1)))
    woT = np.ascontiguousarray(
        np.transpose(wo[:nl] * am[:nl, None, None], (0, 2, 1))
    )  # [L, GLU, H]
    headT = np.ascontiguousarray((head_w * head_rms[None, :] * hls).T)  # [H, VOC]

    stw_pad = np.zeros((384, H), np.float32)
    stw_pad[:CIN] = stem_w.T
    common = {
        "stem_wT": stw_pad,  # [384, H] zero-padded
        "wvT": wvT.astype(bf),
        "wgT": wgT.astype(bf),
        "woT": woT.astype(bf),
        "wl": wl_h.astype(bf),
        "wm": wm_h.astype(bf),
        "headT": headT.astype(bf),
        "ident": np.eye(128, dtype=bf),
        "ones_k": np.ones((128, 1), np.float32),
        "ones_m": np.ones((1, 128), np.float32),
    }
    per_core = []
    for b in range(B):
        xp = np.zeros((384, S), np.float32)
        xp[:CIN] = x[b, :, 0, :]
        per_core.append(dict(common, x=xp))
    return per_core


def run(inputs, n_layers=L, trace=False):
    key = n_layers
    if key not in _PROG_CACHE:
        _PROG_CACHE[key] = build_program(n_layers)
    nc = _PROG_CACHE[key]
    in_maps = _prep_inputs(inputs, n_layers)
    res = run_bass_kernel_spmd(nc, in_maps, core_ids=list(range(B)), trace=trace)
    out = np.stack([r["out"] for r in res.results])  # [B, VOC, S]
    return out[:, :, None, :].astype(np.float32), res


def kernel(**inputs):
    out, _ = run(inputs, L, trace=False)
    return out



# revision 14
# speedup vs baseline: 1.2745x; 1.2745x over previous
"""Trainium2 Bass kernel for nn_ByteGridModel (dense_cnn).

Sharding: pure data-parallel over batch B=8 -> 8 cores, one batch item per
core, no collectives. Weights replicated, streamed one DMA per layer from a
single packed blob (double buffered).

Per-core layout: channels on partitions. The residual stream h
([H=512 -> 4x128, S=256] fp32) lives in 4 PSUM banks so that mixer and
GLU-output matmuls accumulate into it in place (start=False) -- no
separate h-update adds.

Per layer:
  - rmsnorm: ACT square (bf16) -> ones-matmul partition reduction -> ACT
    sqrt -> DVE reciprocal -> broadcast matmul -> DVE multiply (bf16 out).
    rms weights / alphas are folded into the mixer/GLU weights on host.
  - per-channel 16x16 mixers: j-quad DVE/GPSIMD products (all operands
    innermost stride-1 -> 2x DVE mode), accumulated onto h by PE with
    N=256 identity matmuls.
  - GLU MLP: bf16 PE matmuls (Wv/Wg/Wo), Silu + psum->sbuf copy on ACT,
    gate-mul on GPSIMD, wo accumulates onto h. wo matmuls are emitted two
    ot-steps behind p1/p3 to keep the PE stream contiguous.
"""

import numpy as np
import ml_dtypes

import concourse.bacc as bacc
import concourse.bass as bass
import concourse.tile as tile
import concourse.mybir as mybir
from concourse.bass_utils import run_bass_kernel_spmd

B, S, H, GLU, VOC, L, CIN, BLK = 8, 256, 512, 1024, 256, 24, 320, 16
EPS = 1e-5
NT = H // 128  # 4 channel tiles
GT = GLU // 128  # 8 glu tiles

OFF_WV = 0
OFF_WG = 4096
OFF_WO = 8192
OFF_WL = 12288
OFF_WM = 13312
LAYER_SZ = 14336
TAIL_SZ = 1024 + 128  # head (4x256) + ident (128)

F32 = mybir.dt.float32
F32R = mybir.dt.float32r
BF16 = mybir.dt.bfloat16
MULT = mybir.AluOpType.mult
ADD = mybir.AluOpType.add
AF = mybir.ActivationFunctionType

# (tile, quad) product ops assigned to GPSIMD; the rest go to DVE.
# Pool gets late-consumed quads only, so PE never waits on the slow engine.
POOL_QUADS = {(1, 3), (2, 3), (3, 2), (3, 3)}

_PROG_CACHE = {}


def _bview(base, doff, free_dims):
    """View of an sbuf AP with custom (possibly broadcast) free dims."""
    return bass.AP(
        tensor=base.tensor,
        offset=base.offset + doff,
        ap=[list(base.ap[0])] + [list(d) for d in free_dims],
    )


def build_program(n_layers=L, sim_compat=False):
    nc = bacc.Bacc("TRN2", enable_partition_id=False)

    wb_d = nc.dram_tensor(
        "wblob", [128, n_layers * LAYER_SZ + TAIL_SZ], BF16, kind="ExternalInput"
    )
    ms_d = nc.dram_tensor("misc", [384, 768], F32, kind="ExternalInput")
    out_d = nc.dram_tensor("out", [VOC, S], F32, kind="ExternalOutput")

    from contextlib import ExitStack

    with tile.TileContext(nc) as tc, ExitStack() as ctx:
        singles = ctx.enter_context(tc.tile_pool(name="singles", bufs=1))
        wpool = ctx.enter_context(tc.tile_pool(name="wpool", bufs=2))
        npool = ctx.enter_context(tc.tile_pool(name="npool", bufs=2))
        spool = ctx.enter_context(tc.tile_pool(name="spool", bufs=2))
        apool = ctx.enter_context(tc.tile_pool(name="apool", bufs=2))
        ppool = ctx.enter_context(tc.tile_pool(name="ppool", bufs=8))
        gpool = ctx.enter_context(tc.tile_pool(name="gpool", bufs=2))
        ps_h = ctx.enter_context(tc.tile_pool(name="ps_h", bufs=1, space="PSUM"))
        ps_n = ctx.enter_context(tc.tile_pool(name="ps_n", bufs=1, space="PSUM"))
        ps_g = ctx.enter_context(tc.tile_pool(name="ps_g", bufs=3, space="PSUM"))

        # ---- constants / staging ----
        ones_k = singles.tile([128, 1], BF16, tag="ones_k")
        nc.vector.memset(ones_k, 1.0)
        eps_sb = singles.tile([1, 1], F32, tag="eps")
        nc.vector.memset(eps_sb, float(EPS))
        dummy = singles.tile([1, 1], BF16, tag="dummy")

        def preload_act(func):
            # [1,1] activation issued while ACT is idle so the function-set
            # table load happens off the critical chain.
            nc.scalar.activation(dummy, eps_sb, func)

        xs_st = singles.tile([128, 3, 768], F32, tag="xs_st")
        nc.sync.dma_start(out=xs_st, in_=ms_d[:].rearrange("(t p) f -> p t f", p=128))
        xs = singles.tile([128, 3, 768], F32R, tag="xs")
        with nc.allow_low_precision(reason="fp32r staging copy"):
            nc.vector.tensor_copy(out=xs, in_=xs_st)

        tail = singles.tile([128, TAIL_SZ], BF16, tag="tail")
        nc.sync.dma_start(out=tail, in_=wb_d[:, n_layers * LAYER_SZ :])
        ident = tail[:, 1024 : 1024 + 128]
        nc.tensor.ldweights(ident)

        # ---- h tiles resident in PSUM (one bank each) ----
        h = [ps_h.tile([128, S], F32, tag=f"h{t}", name=f"h{t}") for t in range(NT)]

        # ---- stem: h = stem_w @ x ----
        for t in range(NT):
            for kt in range(3):
                nc.tensor.matmul(
                    h[t],
                    xs[:, kt, 256 + t * 128 : 256 + (t + 1) * 128],
                    xs[:, kt, 0:256],
                    start=(kt == 0),
                    stop=(kt == 2),
                )

        def rms_bcast():
            """Returns SBUF [128, S] fp32 broadcast of 1/sqrt(mean(h^2)+eps)."""
            ms = ps_n.tile([1, S], F32, tag="ms")
            for t in range(NT):
                sq = spool.tile([128, S], BF16, tag=f"sq{t}")
                nc.scalar.activation(sq, h[t], AF.Square)
                nc.tensor.matmul(
                    ms,
                    ones_k[:, 0:1],
                    sq[:],
                    start=(t == 0),
                    stop=(t == NT - 1),
                )
            stdv = npool.tile([1, S], F32, tag="stdv")
            nc.scalar.activation(stdv, ms, AF.Sqrt, bias=eps_sb[0:1, 0:1], scale=1.0 / H)
            rstd = npool.tile([1, S], F32, tag="rstd")
            nc.vector.reciprocal(rstd, stdv)
            rb = npool.tile([128, S], F32, tag="rb")
            nc.gpsimd.partition_broadcast(rb[:], rstd[:])
            return rb

        def mixer(wt, local):
            rb = rms_bcast()
            un = []
            for t in range(NT):
                u = apool.tile([128, S], BF16, tag=f"u{t}", name=f"u{t}")
                if local:
                    nc.vector.tensor_tensor(out=u, in0=h[t], in1=rb, op=MULT)
                else:
                    # transposed write: u[c, 16j+i] = h[c,16i+j] * rb[16i+j]
                    inv_h = _bview(h[t][:], 0, [[16, 16], [1, 16]])
                    inv_rb = _bview(rb[:], 0, [[16, 16], [1, 16]])
                    outv = _bview(u[:], 0, [[1, 16], [16, 16]])
                    nc.vector.tensor_tensor(out=outv, in0=inv_h, in1=inv_rb, op=MULT)
                un.append(u)
            woff = OFF_WL if local else OFF_WM
            # all products first (engine-split), then the PE accumulation
            # per tile so PE consumes a fully-banked stream.
            prods = {}
            for t in range(NT):
                for q in range(4):
                    pr = ppool.tile([128, 16, 16, 4], BF16, tag="prod", name="pr")
                    if local:
                        # prod[c,i,p,jq] = u[c,16i+4q+jq] * wl[c,16p+4q+jq]
                        uv = _bview(un[t][:], 4 * q, [[16, 16], [0, 16], [1, 4]])
                        wv = _bview(
                            wt[:], woff + t * 256 + 4 * q, [[0, 16], [16, 16], [1, 4]]
                        )
                    else:
                        # prod[c,p,j,iq] = vT[c,16j+4q+iq] * wm[c,16p+4q+iq]
                        uv = _bview(un[t][:], 4 * q, [[0, 16], [16, 16], [1, 4]])
                        wv = _bview(
                            wt[:], woff + t * 256 + 4 * q, [[16, 16], [0, 16], [1, 4]]
                        )
                    ov = _bview(pr[:], 0, [[64, 16], [4, 16], [1, 4]])
                    eng = nc.gpsimd if (t, q) in POOL_QUADS else nc.vector
                    eng.tensor_tensor(out=ov, in0=uv, in1=wv, op=MULT)
                    prods[(t, q)] = pr
            for t in range(NT):
                for q in range(4):
                    for jj in range(4):
                        mov = _bview(prods[(t, q)][:], jj, [[64, 16], [4, 16]])
                        nc.tensor.matmul(
                            h[t],
                            ident,
                            mov,
                            start=False,
                            stop=(q == 3 and jj == 3),
                        )

        for l in range(n_layers):
            wt = wpool.tile([128, LAYER_SZ], BF16, tag="wt", name="wt")
            nc.sync.dma_start(out=wt, in_=wb_d[:, l * LAYER_SZ : (l + 1) * LAYER_SZ])
            nc.tensor.ldweights(wt[:, 0:128])

            mixer(wt, local=True)
            mixer(wt, local=False)

            # ---------- GLU MLP
            rb = rms_bcast()
            # preload the silu table while DVE computes wn / PE runs p1
            if not sim_compat:
                preload_act(AF.Silu)
            wn = []
            for t in range(NT):
                w = apool.tile([128, S], BF16, tag=f"wn{t}", name=f"wn{t}")
                nc.vector.tensor_tensor(out=w, in0=h[t], in1=rb, op=MULT)
                wn.append(w)

            gts = []
            # interleave: p1(ot), p3(ot), then wo for gt[ot-2]
            for ot in range(GT):
                p1 = ps_g.tile([128, S], F32, tag="pg")
                for kt in range(NT):
                    nc.tensor.matmul(
                        p1,
                        wt[:, OFF_WV + kt * 1024 + ot * 128 : OFF_WV + kt * 1024 + (ot + 1) * 128],
                        wn[kt][:],
                        start=(kt == 0),
                        stop=(kt == NT - 1),
                    )
                s1 = apool.tile([128, S], BF16, tag="s1", name="s1")
                if sim_compat:
                    sg = apool.tile([128, S], BF16, tag="sg", name="sg")
                    nc.scalar.activation(sg, p1, AF.Sigmoid)
                    nc.vector.tensor_tensor(out=s1, in0=sg, in1=p1, op=MULT)
                else:
                    nc.scalar.activation(s1, p1, AF.Silu)
                p3 = ps_g.tile([128, S], F32, tag="pg")
                for kt in range(NT):
                    nc.tensor.matmul(
                        p3,
                        wt[:, OFF_WG + kt * 1024 + ot * 128 : OFF_WG + kt * 1024 + (ot + 1) * 128],
                        wn[kt][:],
                        start=(kt == 0),
                        stop=(kt == NT - 1),
                    )
                gt_ = gpool.tile([128, S], BF16, tag=f"g{ot}", name="gt_")
                nc.vector.tensor_tensor(out=gt_, in0=s1, in1=p3, op=MULT)
                gts.append(gt_)
                if ot == GT - 1:
                    # preload the sqrt table while PE finishes the wo tail
                    preload_act(AF.Sqrt)
                if ot >= 2:
                    go = ot - 2
                    for t in range(NT):
                        nc.tensor.matmul(
                            h[t],
                            wt[:, OFF_WO + go * 512 + t * 128 : OFF_WO + go * 512 + (t + 1) * 128],
                            gts[go][:],
                            start=False,
                            stop=False,
                        )
            for go in (GT - 2, GT - 1):
                for t in range(NT):
                    nc.tensor.matmul(
                        h[t],
                        wt[:, OFF_WO + go * 512 + t * 128 : OFF_WO + go * 512 + (t + 1) * 128],
                        gts[go][:],
                        start=False,
                        stop=(go == GT - 1),
                    )

        # ---------- head ----------
        rb = rms_bcast()
        nrm = []
        for t in range(NT):
            n_ = apool.tile([128, S], BF16, tag=f"wn{t}", name=f"n_{t}")
            nc.vector.tensor_tensor(out=n_, in0=h[t], in1=rb, op=MULT)
            nrm.append(n_)
        osb = singles.tile([128, 2, S], F32, tag="osb")
        for mc in range(VOC // 128):
            po = ps_g.tile([128, S], F32, tag="pg")
            for kt in range(NT):
                nc.tensor.matmul(
                    po,
                    tail[:, kt * 256 + mc * 128 : kt * 256 + (mc + 1) * 128],
                    nrm[kt][:],
                    start=(kt == 0),
                    stop=(kt == NT - 1),
                )
            nc.vector.tensor_copy(out=osb[:, mc], in_=po)
        nc.sync.dma_start(
            out=out_d[:].rearrange("(t p) s -> p t s", p=128), in_=osb
        )

    nc.compile()
    return nc


def _prep_inputs(inputs, n_layers=L):
    """Host-side weight folding + blob packing. Returns per-core input dicts."""
    f = lambda k: np.asarray(inputs[k], dtype=np.float32)
    x = f("x")
    stem_w = f("stem_w")  # [H, CIN]
    rl, rg, rf = f("rms_local"), f("rms_global"), f("rms_ffn")
    al, ag, am = f("alpha_local"), f("alpha_global"), f("alpha_mlp")
    w_local, w_global = f("w_local"), f("w_global")  # [L, H, BLK, BLK]
    wv, wg, wo = f("wv"), f("wg"), f("wo")
    head_rms, head_w = f("head_rms"), f("head_w")
    hls = np.float32(np.asarray(inputs["head_logit_scale"]))

    bf = ml_dtypes.bfloat16
    nl = n_layers

    wl_h = (w_local[:nl] * al[:nl, None, None, None] * rl[:nl, :, None, None]).reshape(
        nl, H, 256
    )
    wm_h = (w_global[:nl] * ag[:nl, None, None, None] * rg[:nl, :, None, None]).reshape(
        nl, H, 256
    )
    wvT = np.transpose(wv[:nl] * rf[:nl, None, :], (0, 2, 1))  # [L, H, GLU]
    wgT = np.transpose(wg[:nl] * rf[:nl, None, :], (0, 2, 1))
    woT = np.transpose(wo[:nl] * am[:nl, None, None], (0, 2, 1))  # [L, GLU, H]
    headT = (head_w * head_rms[None, :] * hls).T  # [H, VOC]

    parts = []
    for l in range(nl):
        parts.append(wvT[l].reshape(NT, 128, GLU).transpose(1, 0, 2).reshape(128, -1))
        parts.append(wgT[l].reshape(NT, 128, GLU).transpose(1, 0, 2).reshape(128, -1))
        parts.append(woT[l].reshape(GT, 128, H).transpose(1, 0, 2).reshape(128, -1))
        parts.append(wl_h[l].reshape(NT, 128, 256).transpose(1, 0, 2).reshape(128, -1))
        parts.append(wm_h[l].reshape(NT, 128, 256).transpose(1, 0, 2).reshape(128, -1))
    parts.append(headT.reshape(NT, 128, VOC).transpose(1, 0, 2).reshape(128, -1))
    parts.append(np.eye(128, dtype=np.float32))
    wblob = np.ascontiguousarray(np.concatenate(parts, axis=1)).astype(bf)

    stem_pad = np.zeros((384, H), np.float32)
    stem_pad[:CIN] = stem_w.T
    per_core = []
    for b in range(B):
        misc = np.zeros((384, 768), np.float32)
        misc[:CIN, 0:256] = x[b, :, 0, :]
        misc[:, 256:768] = stem_pad
        per_core.append({"wblob": wblob, "misc": misc})
    return per_core


def run(inputs, n_layers=L, trace=False):
    key = n_layers
    if key not in _PROG_CACHE:
        _PROG_CACHE[key] = build_program(n_layers)
    nc = _PROG_CACHE[key]
    in_maps = _prep_inputs(inputs, n_layers)
    res = run_bass_kernel_spmd(nc, in_maps, core_ids=list(range(B)), trace=trace)
    out = np.stack([r["out"] for r in res.results])  # [B, VOC, S]
    return out[:, :, None, :].astype(np.float32), res


def kernel(**inputs):
    out, _ = run(inputs, L, trace=False)
    return out


# revision 19
# speedup vs baseline: 1.5596x; 1.2238x over previous
"""Trainium2 Bass kernel for nn_ByteGridModel (dense_cnn).

Sharding: pure data-parallel over batch B=8 -> 8 cores, one batch item per
core, no collectives. Weights replicated, streamed one DMA per layer from a
single packed blob (double buffered).

Per-core layout: channels on partitions. The residual stream h
([H=512 -> 4x128, S=256] fp32) lives in 4 PSUM banks so that mixer and
GLU-output matmuls accumulate into it in place (start=False) -- no
separate h-update adds.

Per layer:
  - rmsnorm: ACT square (bf16) -> ones-matmul partition reduction -> ACT
    sqrt -> DVE reciprocal -> broadcast matmul -> DVE multiply (bf16 out).
    rms weights / alphas are folded into the mixer/GLU weights on host.
  - per-channel 16x16 mixers: j-quad DVE/GPSIMD products (all operands
    innermost stride-1 -> 2x DVE mode), accumulated onto h by PE with
    N=256 identity matmuls.
  - GLU MLP: bf16 PE matmuls (Wv/Wg/Wo), Silu + psum->sbuf copy on ACT,
    gate-mul on GPSIMD, wo accumulates onto h. wo matmuls are emitted two
    ot-steps behind p1/p3 to keep the PE stream contiguous.
"""

import numpy as np
import ml_dtypes

import concourse.bacc as bacc
import concourse.bass as bass
import concourse.tile as tile
import concourse.mybir as mybir
from concourse.bass_utils import run_bass_kernel_spmd

B, S, H, GLU, VOC, L, CIN, BLK = 8, 256, 512, 1024, 256, 24, 320, 16
EPS = 1e-5
NT = H // 128  # 4 channel tiles
GT = GLU // 128  # 8 glu tiles

OFF_WV = 0
OFF_WG = 4096
OFF_WO = 8192
OFF_WL = 12288
OFF_WM = 13312
LAYER_SZ = 14336
TAIL_SZ = 1024 + 128  # head (4x256) + ident (128)

F32 = mybir.dt.float32
F32R = mybir.dt.float32r
BF16 = mybir.dt.bfloat16
MULT = mybir.AluOpType.mult
ADD = mybir.AluOpType.add
AF = mybir.ActivationFunctionType

# (tile, quad) product ops assigned to GPSIMD; the rest go to DVE.
# Pool gets late-consumed quads only, so PE never waits on the slow engine.
POOL_QUADS = {(1, 3), (2, 3), (3, 2), (3, 3)}

_PROG_CACHE = {}


def _bview(base, doff, free_dims):
    """View of an sbuf AP with custom (possibly broadcast) free dims."""
    return bass.AP(
        tensor=base.tensor,
        offset=base.offset + doff,
        ap=[list(base.ap[0])] + [list(d) for d in free_dims],
    )


def build_program(n_layers=L, sim_compat=False):
    nc = bacc.Bacc("TRN2", enable_partition_id=False)

    wb_d = nc.dram_tensor(
        "wblob", [128, n_layers * LAYER_SZ + TAIL_SZ], BF16, kind="ExternalInput"
    )
    ms_d = nc.dram_tensor("misc", [384, 768], F32, kind="ExternalInput")
    out_d = nc.dram_tensor("out", [VOC, S], F32, kind="ExternalOutput")

    from contextlib import ExitStack

    with tile.TileContext(nc) as tc, ExitStack() as ctx:
        singles = ctx.enter_context(tc.tile_pool(name="singles", bufs=1))
        wpool = ctx.enter_context(tc.tile_pool(name="wpool", bufs=2))
        npool = ctx.enter_context(tc.tile_pool(name="npool", bufs=2))
        spool = ctx.enter_context(tc.tile_pool(name="spool", bufs=2))
        apool = ctx.enter_context(tc.tile_pool(name="apool", bufs=4))
        ppool = ctx.enter_context(tc.tile_pool(name="ppool", bufs=8))
        gpool = ctx.enter_context(tc.tile_pool(name="gpool", bufs=2))
        ps_h = ctx.enter_context(tc.tile_pool(name="ps_h", bufs=1, space="PSUM"))
        ps_n = ctx.enter_context(tc.tile_pool(name="ps_n", bufs=1, space="PSUM"))
        ps_g = ctx.enter_context(tc.tile_pool(name="ps_g", bufs=3, space="PSUM"))

        # ---- constants / staging ----
        ones_k = singles.tile([128, 1], BF16, tag="ones_k")
        nc.vector.memset(ones_k, 1.0)
        ones_m_f = singles.tile([1, 128], F32, tag="ones_m_f")
        nc.vector.memset(ones_m_f, 1.0)
        ones_m = singles.tile([1, 128], F32R, tag="ones_m")
        with nc.allow_low_precision(reason="fp32r ones"):
            nc.vector.tensor_copy(out=ones_m, in_=ones_m_f)
        eps_sb = singles.tile([1, 1], F32, tag="eps")
        nc.vector.memset(eps_sb, float(EPS))
        dummy = singles.tile([1, 1], BF16, tag="dummy")

        def preload_act(func):
            # [1,1] activation issued while ACT is idle so the function-set
            # table load happens off the critical chain.
            nc.scalar.activation(dummy, eps_sb, func)

        xs_st = singles.tile([128, 3, 768], F32, tag="xs_st")
        nc.sync.dma_start(out=xs_st, in_=ms_d[:].rearrange("(t p) f -> p t f", p=128))
        xs = singles.tile([128, 3, 768], F32R, tag="xs")
        with nc.allow_low_precision(reason="fp32r staging copy"):
            nc.vector.tensor_copy(out=xs, in_=xs_st)

        tail = singles.tile([128, TAIL_SZ], BF16, tag="tail")
        nc.sync.dma_start(out=tail, in_=wb_d[:, n_layers * LAYER_SZ :])
        ident = tail[:, 1024 : 1024 + 128]
        nc.tensor.ldweights(ident)

        # ---- h tiles resident in PSUM (one bank each) ----
        h = [ps_h.tile([128, S], F32, tag=f"h{t}", name=f"h{t}") for t in range(NT)]

        # ---- stem: h = stem_w @ x ----
        for t in range(NT):
            for kt in range(3):
                nc.tensor.matmul(
                    h[t],
                    xs[:, kt, 256 + t * 128 : 256 + (t + 1) * 128],
                    xs[:, kt, 0:256],
                    start=(kt == 0),
                    stop=(kt == 2),
                )

        def rms_bcast():
            """Returns SBUF [128, S] fp32 broadcast of 1/sqrt(mean(h^2)+eps).
            ms shares the rb psum bank ([0:1] slice); the PE broadcast matmul
            keeps the tensor engine warm through the rms window and an ACT
            copy moves rb to SBUF so u-mults have only one PSUM operand."""
            rbms = ps_n.tile([128, S], F32, tag="rbms")
            ms = rbms[0:1, :]
            for t in range(NT):
                sq = spool.tile([128, S], BF16, tag=f"sq{t}")
                nc.scalar.activation(sq, h[t], AF.Square)
                nc.tensor.matmul(
                    ms,
                    ones_k[:, 0:1],
                    sq[:],
                    start=(t == 0),
                    stop=(t == NT - 1),
                )
            stdv = npool.tile([1, S], F32, tag="stdv")
            nc.scalar.activation(stdv, ms, AF.Sqrt, bias=eps_sb[0:1, 0:1], scale=1.0 / H)
            rstd = npool.tile([1, S], F32R, tag="rstd")
            with nc.allow_low_precision(reason="fp32r rstd for broadcast matmul"):
                nc.vector.reciprocal(rstd, stdv)
            nc.tensor.matmul(rbms, ones_m[0:1, :], rstd[:], start=True, stop=True)
            rb = npool.tile([128, S], F32, tag="rb")
            nc.scalar.activation(rb, rbms, AF.Copy)
            return rb

        def mixer(wt, local):
            rb = rms_bcast()
            un = []
            for t in range(NT):
                u = apool.tile([128, S], BF16, tag=f"u{t}", name=f"u{t}")
                if local:
                    nc.vector.tensor_tensor(out=u, in0=h[t], in1=rb, op=MULT)
                else:
                    # transposed write: u[c, 16j+i] = h[c,16i+j] * rb[16i+j]
                    inv_h = _bview(h[t][:], 0, [[16, 16], [1, 16]])
                    inv_rb = _bview(rb[:], 0, [[16, 16], [1, 16]])
                    outv = _bview(u[:], 0, [[1, 16], [16, 16]])
                    nc.vector.tensor_tensor(out=outv, in0=inv_h, in1=inv_rb, op=MULT)
                un.append(u)
            woff = OFF_WL if local else OFF_WM
            # all products first (engine-split), then the PE accumulation
            # per tile so PE consumes a fully-banked stream.
            prods = {}
            for t in range(NT):
                for q in range(4):
                    pr = ppool.tile([128, 16, 16, 4], BF16, tag="prod", name="pr")
                    if local:
                        # prod[c,i,p,jq] = u[c,16i+4q+jq] * wl[c,16p+4q+jq]
                        uv = _bview(un[t][:], 4 * q, [[16, 16], [0, 16], [1, 4]])
                        wv = _bview(
                            wt[:], woff + t * 256 + 4 * q, [[0, 16], [16, 16], [1, 4]]
                        )
                    else:
                        # prod[c,p,j,iq] = vT[c,16j+4q+iq] * wm[c,16p+4q+iq]
                        uv = _bview(un[t][:], 4 * q, [[0, 16], [16, 16], [1, 4]])
                        wv = _bview(
                            wt[:], woff + t * 256 + 4 * q, [[16, 16], [0, 16], [1, 4]]
                        )
                    ov = _bview(pr[:], 0, [[64, 16], [4, 16], [1, 4]])
                    eng = nc.gpsimd if (t, q) in POOL_QUADS else nc.vector
                    eng.tensor_tensor(out=ov, in0=uv, in1=wv, op=MULT)
                    prods[(t, q)] = pr
            for t in range(NT):
                for q in range(4):
                    for jj in range(4):
                        mov = _bview(prods[(t, q)][:], jj, [[64, 16], [4, 16]])
                        nc.tensor.matmul(
                            h[t],
                            ident,
                            mov,
                            start=False,
                            stop=(q == 3 and jj == 3),
                        )

        for l in range(n_layers):
            wt = wpool.tile([128, LAYER_SZ], BF16, tag="wt", name="wt")
            nc.sync.dma_start(out=wt, in_=wb_d[:, l * LAYER_SZ : (l + 1) * LAYER_SZ])
            nc.tensor.ldweights(wt[:, 0:128])

            mixer(wt, local=True)
            mixer(wt, local=False)

            # ---------- GLU MLP
            rb = rms_bcast()
            # preload the silu table while DVE computes wn / PE runs p1
            if not sim_compat:
                preload_act(AF.Silu)
            wn = []
            for t in range(NT):
                w = apool.tile([128, S], BF16, tag=f"wn{t}", name=f"wn{t}")
                nc.vector.tensor_tensor(out=w, in0=h[t], in1=rb, op=MULT)
                wn.append(w)

            gts = []
            # interleave: p1(ot), p3(ot), then wo for gt[ot-2]
            for ot in range(GT):
                p1 = ps_g.tile([128, S], F32, tag="pg")
                for kt in range(NT):
                    nc.tensor.matmul(
                        p1,
                        wt[:, OFF_WV + kt * 1024 + ot * 128 : OFF_WV + kt * 1024 + (ot + 1) * 128],
                        wn[kt][:],
                        start=(kt == 0),
                        stop=(kt == NT - 1),
                    )
                s1 = apool.tile([128, S], BF16, tag="s1", name="s1")
                if sim_compat:
                    sg = apool.tile([128, S], BF16, tag="sg", name="sg")
                    nc.scalar.activation(sg, p1, AF.Sigmoid)
                    nc.vector.tensor_tensor(out=s1, in0=sg, in1=p1, op=MULT)
                else:
                    nc.scalar.activation(s1, p1, AF.Silu)
                # odd p3s borrow the rms broadcast bank (idle during GLU) so
                # the p1/p3 rotation spans 4 banks instead of 3.
                if ot % 2 == 1:
                    p3 = ps_n.tile([128, S], F32, tag="rbms")
                else:
                    p3 = ps_g.tile([128, S], F32, tag="pg")
                for kt in range(NT):
                    nc.tensor.matmul(
                        p3,
                        wt[:, OFF_WG + kt * 1024 + ot * 128 : OFF_WG + kt * 1024 + (ot + 1) * 128],
                        wn[kt][:],
                        start=(kt == 0),
                        stop=(kt == NT - 1),
                    )
                gt_ = gpool.tile([128, S], BF16, tag=f"g{ot}", name="gt_")
                nc.vector.tensor_tensor(out=gt_, in0=s1, in1=p3, op=MULT)
                gts.append(gt_)
                if ot == GT - 1:
                    # preload the sqrt table while PE finishes the wo tail
                    preload_act(AF.Sqrt)
                if ot >= 2:
                    go = ot - 2
                    for t in range(NT):
                        nc.tensor.matmul(
                            h[t],
                            wt[:, OFF_WO + go * 512 + t * 128 : OFF_WO + go * 512 + (t + 1) * 128],
                            gts[go][:],
                            start=False,
                            stop=False,
                        )
            for go in (GT - 2, GT - 1):
                for t in range(NT):
                    nc.tensor.matmul(
                        h[t],
                        wt[:, OFF_WO + go * 512 + t * 128 : OFF_WO + go * 512 + (t + 1) * 128],
                        gts[go][:],
                        start=False,
                        stop=(go == GT - 1),
                    )

        # ---------- head ----------
        rb = rms_bcast()
        nrm = []
        for t in range(NT):
            n_ = apool.tile([128, S], BF16, tag=f"wn{t}", name=f"n_{t}")
            nc.vector.tensor_tensor(out=n_, in0=h[t], in1=rb, op=MULT)
            nrm.append(n_)
        osb = singles.tile([128, 2, S], F32, tag="osb")
        for mc in range(VOC // 128):
            po = ps_g.tile([128, S], F32, tag="pg")
            for kt in range(NT):
                nc.tensor.matmul(
                    po,
                    tail[:, kt * 256 + mc * 128 : kt * 256 + (mc + 1) * 128],
                    nrm[kt][:],
                    start=(kt == 0),
                    stop=(kt == NT - 1),
                )
            nc.vector.tensor_copy(out=osb[:, mc], in_=po)
        nc.sync.dma_start(
            out=out_d[:].rearrange("(t p) s -> p t s", p=128), in_=osb
        )

    nc.compile()
    return nc


def _prep_inputs(inputs, n_layers=L):
    """Host-side weight folding + blob packing. Returns per-core input dicts."""
    f = lambda k: np.asarray(inputs[k], dtype=np.float32)
    x = f("x")
    stem_w = f("stem_w")  # [H, CIN]
    rl, rg, rf = f("rms_local"), f("rms_global"), f("rms_ffn")
    al, ag, am = f("alpha_local"), f("alpha_global"), f("alpha_mlp")
    w_local, w_global = f("w_local"), f("w_global")  # [L, H, BLK, BLK]
    wv, wg, wo = f("wv"), f("wg"), f("wo")
    head_rms, head_w = f("head_rms"), f("head_w")
    hls = np.float32(np.asarray(inputs["head_logit_scale"]))

    bf = ml_dtypes.bfloat16
    nl = n_layers

    wl_h = (w_local[:nl] * al[:nl, None, None, None] * rl[:nl, :, None, None]).reshape(
        nl, H, 256
    )
    wm_h = (w_global[:nl] * ag[:nl, None, None, None] * rg[:nl, :, None, None]).reshape(
        nl, H, 256
    )
    wvT = np.transpose(wv[:nl] * rf[:nl, None, :], (0, 2, 1))  # [L, H, GLU]
    wgT = np.transpose(wg[:nl] * rf[:nl, None, :], (0, 2, 1))
    woT = np.transpose(wo[:nl] * am[:nl, None, None], (0, 2, 1))  # [L, GLU, H]
    headT = (head_w * head_rms[None, :] * hls).T  # [H, VOC]

    parts = []
    for l in range(nl):
        parts.append(wvT[l].reshape(NT, 128, GLU).transpose(1, 0, 2).reshape(128, -1))
        parts.append(wgT[l].reshape(NT, 128, GLU).transpose(1, 0, 2).reshape(128, -1))
        parts.append(woT[l].reshape(GT, 128, H).transpose(1, 0, 2).reshape(128, -1))
        parts.append(wl_h[l].reshape(NT, 128, 256).transpose(1, 0, 2).reshape(128, -1))
        parts.append(wm_h[l].reshape(NT, 128, 256).transpose(1, 0, 2).reshape(128, -1))
    parts.append(headT.reshape(NT, 128, VOC).transpose(1, 0, 2).reshape(128, -1))
    parts.append(np.eye(128, dtype=np.float32))
    wblob = np.ascontiguousarray(np.concatenate(parts, axis=1)).astype(bf)

    stem_pad = np.zeros((384, H), np.float32)
    stem_pad[:CIN] = stem_w.T
    per_core = []
    for b in range(B):
        misc = np.zeros((384, 768), np.float32)
        misc[:CIN, 0:256] = x[b, :, 0, :]
        misc[:, 256:768] = stem_pad
        per_core.append({"wblob": wblob, "misc": misc})
    return per_core


def run(inputs, n_layers=L, trace=False):
    key = n_layers
    if key not in _PROG_CACHE:
        _PROG_CACHE[key] = build_program(n_layers)
    nc = _PROG_CACHE[key]
    in_maps = _prep_inputs(inputs, n_layers)
    res = run_bass_kernel_spmd(nc, in_maps, core_ids=list(range(B)), trace=trace)
    out = np.stack([r["out"] for r in res.results])  # [B, VOC, S]
    return out[:, :, None, :].astype(np.float32), res


def kernel(**inputs):
    out, _ = run(inputs, L, trace=False)
    return out
